# revision 7
# baseline (speedup 1.0000x reference)
import sys

for _p in (
    "/root/.axon_site",
    "/root/.axon_site/_ro/trn_rl_repo",
    "/root/.axon_site/_ro/pypackages",
    "/opt/trn_rl_repo",
):
    if _p not in sys.path:
        sys.path.append(_p)

import numpy as np

B, C, H, W = 4, 64, 256, 256
K = 3
T = K * K
WO = W - K + 1
HO = H - K + 1
NPLANES = B * C
NCORES = 8
ROWS = 32
R = 4
KR = ROWS + K - 1
NBLK = ROWS // R
NGRP = NPLANES // 128

_CACHE = {}


def _build_nc():
    import concourse.bass as bass
    import concourse.mybir as mybir
    from concourse import bacc
    from concourse.tile import TileContext

    f32 = mybir.dt.float32
    nc = bacc.Bacc("TRN2", target_bir_lowering=False, debug=False, num_devices=NCORES)
    key = nc.declare_dram_parameter("key", [NPLANES, KR * W], f32, isOutput=False)
    query = nc.declare_dram_parameter("query", [NPLANES, ROWS * W], f32, isOutput=False)
    out = nc.declare_dram_parameter("out", [NPLANES, ROWS * WO * T], f32, isOutput=True)

    with TileContext(nc) as tc:
        with (
            tc.tile_pool(name="kq", bufs=3) as kq_pool,
            tc.tile_pool(name="op", bufs=4) as out_pool,
        ):
            for g in range(NGRP):
                ktiles = {}

                def _load_key(blk, g, store):
                    # last block of the group carries its own 2-row halo;
                    # earlier blocks borrow halo rows from the next tile
                    nrows = R + 2 if blk == NBLK - 1 else R
                    t = kq_pool.tile([128, nrows * W], f32, tag="key")
                    r0 = blk * R
                    nc.scalar.dma_start(
                        out=t[:],
                        in_=key[g * 128:(g + 1) * 128, r0 * W:(r0 + nrows) * W],
                    )
                    store[blk] = t

                _load_key(0, g, ktiles)

                for blk in range(NBLK):
                    if blk + 1 < NBLK:
                        _load_key(blk + 1, g, ktiles)
                    r0 = blk * R
                    qtile = kq_pool.tile([128, R * W], f32, tag="query")
                    nc.scalar.dma_start(
                        out=qtile[:],
                        in_=query[g * 128:(g + 1) * 128, r0 * W:(r0 + R) * W],
                    )
                    otile = out_pool.tile([128, R * WO * T], f32, tag="out")
                    qv = qtile[:].rearrange("p (r w) -> p r w", w=W)
                    ov = otile[:].rearrange(
                        "p (r w kh kw) -> p r w kh kw", w=WO, kh=K, kw=K
                    )
                    own = ktiles[blk][:]
                    nxt = ktiles[blk + 1][:] if blk + 1 < NBLK else None
                    own_rows = R + 2 if blk == NBLK - 1 else R

                    def emit(rlo, rhi):
                        # multiply taps for output rows [rlo, rhi) of this blk
                        for kh in range(K):
                            # rows with r + kh < own_rows come from own tile
                            cut = min(rhi, max(rlo, own_rows - kh))
                            for lo, hi, tile_ap, base in (
                                (rlo, cut, own, kh),
                                (cut, rhi, nxt, kh - own_rows),
                            ):
                                if hi <= lo:
                                    continue
                                kap = bass.AP(
                                    tensor=tile_ap.tensor,
                                    offset=(lo + base) * W,
                                    ap=[
                                        list(tile_ap.ap[0]),
                                        [W, hi - lo],
                                        [1, WO],
                                        [1, K],
                                    ],
                                )
                                qb = (
                                    qv[:, lo:hi, 1:1 + WO]
                                    .unsqueeze(3)
                                    .to_broadcast((128, hi - lo, WO, K))
                                )
                                nc.vector.tensor_mul(
                                    ov[:, lo:hi, :, kh, :], kap, qb
                                )

                    first = g == 0 and blk == 0
                    last = g == NGRP - 1 and blk == NBLK - 1
                    sub = R if (first or last) else 1
                    rstep = R // sub
                    for s in range(sub):
                        rs = s * rstep
                        emit(rs, rs + rstep)
                        go = (r0 + rs) * WO * T
                        nc.sync.dma_start(
                            out=out[
                                g * 128:(g + 1) * 128, go:go + rstep * WO * T
                            ],
                            in_=otile[:, rs * WO * T:(rs + rstep) * WO * T],
                        )
                    del ktiles[blk]
    nc.compile()
    return nc


def _get_nc():
    if "nc" not in _CACHE:
        _CACHE["nc"] = _build_nc()
    return _CACHE["nc"]


def _make_in_maps(key_map, query_map):
    kflat = np.ascontiguousarray(key_map.reshape(NPLANES, H, W))
    qflat = np.ascontiguousarray(query_map.reshape(NPLANES, H, W))
    in_maps = []
    for i in range(NCORES):
        r0 = ROWS * i
        kshard = np.zeros((NPLANES, KR, W), np.float32)
        nrows = min(KR, H - r0)
        kshard[:, :nrows] = kflat[:, r0:r0 + nrows]
        qshard = np.zeros((NPLANES, ROWS, W), np.float32)
        qrows = min(ROWS, H - (r0 + 1))
        qshard[:, :qrows] = qflat[:, r0 + 1:r0 + 1 + qrows]
        in_maps.append({
            "key": kshard.reshape(NPLANES, KR * W),
            "query": qshard.reshape(NPLANES, ROWS * W),
        })
    return in_maps


def run_spmd(key_map, query_map, trace=False, **kwargs):
    from concourse.bass_utils import run_bass_kernel_spmd

    nc = _get_nc()
    in_maps = _make_in_maps(key_map, query_map)
    res = run_bass_kernel_spmd(
        nc, in_maps, core_ids=list(range(NCORES)), trace=trace, **kwargs
    )
    outs = [res.results[i]["out"].reshape(NPLANES, ROWS, WO, K, K)
            for i in range(NCORES)]
    full = np.concatenate(outs, axis=1)[:, :HO]
    return full.reshape(B, C, HO * WO, K, K), res


def kernel(key_map, query_map, k, stride):
    assert int(k) == K and int(stride) == 1
    key_map = np.asarray(key_map, dtype=np.float32)
    query_map = np.asarray(query_map, dtype=np.float32)
    out, _ = run_spmd(key_map, query_map, trace=False)
    return out


# revision 8
# speedup vs baseline: 1.0097x; 1.0097x over previous
import sys

for _p in (
    "/root/.axon_site",
    "/root/.axon_site/_ro/trn_rl_repo",
    "/root/.axon_site/_ro/pypackages",
    "/opt/trn_rl_repo",
):
    if _p not in sys.path:
        sys.path.append(_p)

import numpy as np

B, C, H, W = 4, 64, 256, 256
K = 3
T = K * K
WO = W - K + 1
HO = H - K + 1
NPLANES = B * C
NCORES = 8
ROWS = 32
R = 4
KR = ROWS + K - 1
NBLK = ROWS // R
NGRP = NPLANES // 128

_CACHE = {}


def _build_nc():
    import concourse.bass as bass
    import concourse.mybir as mybir
    from concourse import bacc
    from concourse.tile import TileContext

    f32 = mybir.dt.float32
    nc = bacc.Bacc("TRN2", target_bir_lowering=False, debug=False, num_devices=NCORES)
    key = nc.declare_dram_parameter("key", [NPLANES, KR * W], f32, isOutput=False)
    query = nc.declare_dram_parameter("query", [NPLANES, ROWS * W], f32, isOutput=False)
    out = nc.declare_dram_parameter("out", [NPLANES, ROWS * WO * T], f32, isOutput=True)

    with TileContext(nc) as tc:
        with (
            tc.tile_pool(name="kq", bufs=4) as kq_pool,
            tc.tile_pool(name="op", bufs=4) as out_pool,
        ):
            for g in range(NGRP):
                ktiles = {}

                def _load_key(blk, g, store):
                    # last block of the group carries its own 2-row halo;
                    # earlier blocks borrow halo rows from the next tile
                    nrows = R + 2 if blk == NBLK - 1 else R
                    t = kq_pool.tile([128, nrows * W], f32, tag="key")
                    r0 = blk * R
                    nc.scalar.dma_start(
                        out=t[:],
                        in_=key[g * 128:(g + 1) * 128, r0 * W:(r0 + nrows) * W],
                    )
                    store[blk] = t

                _load_key(0, g, ktiles)

                for blk in range(NBLK):
                    if blk + 1 < NBLK:
                        _load_key(blk + 1, g, ktiles)
                    r0 = blk * R
                    qtile = kq_pool.tile([128, R * W], f32, tag="query")
                    nc.scalar.dma_start(
                        out=qtile[:],
                        in_=query[g * 128:(g + 1) * 128, r0 * W:(r0 + R) * W],
                    )
                    otile = out_pool.tile([128, R * WO * T], f32, tag="out")
                    qv = qtile[:].rearrange("p (r w) -> p r w", w=W)
                    ov = otile[:].rearrange(
                        "p (r w kh kw) -> p r w kh kw", w=WO, kh=K, kw=K
                    )
                    own = ktiles[blk][:]
                    nxt = ktiles[blk + 1][:] if blk + 1 < NBLK else None
                    own_rows = R + 2 if blk == NBLK - 1 else R

                    def emit(rlo, rhi):
                        # multiply taps for output rows [rlo, rhi) of this blk
                        for kh in range(K):
                            # rows with r + kh < own_rows come from own tile
                            cut = min(rhi, max(rlo, own_rows - kh))
                            for lo, hi, tile_ap, base in (
                                (rlo, cut, own, kh),
                                (cut, rhi, nxt, kh - own_rows),
                            ):
                                if hi <= lo:
                                    continue
                                kap = bass.AP(
                                    tensor=tile_ap.tensor,
                                    offset=(lo + base) * W,
                                    ap=[
                                        list(tile_ap.ap[0]),
                                        [W, hi - lo],
                                        [1, WO],
                                        [1, K],
                                    ],
                                )
                                qb = (
                                    qv[:, lo:hi, 1:1 + WO]
                                    .unsqueeze(3)
                                    .to_broadcast((128, hi - lo, WO, K))
                                )
                                nc.vector.tensor_mul(
                                    ov[:, lo:hi, :, kh, :], kap, qb
                                )

                    first = g == 0 and blk == 0
                    last = g == NGRP - 1 and blk == NBLK - 1
                    sub = R if (first or last) else 1
                    rstep = R // sub
                    for s in range(sub):
                        rs = s * rstep
                        emit(rs, rs + rstep)
                        go = (r0 + rs) * WO * T
                        nc.sync.dma_start(
                            out=out[
                                g * 128:(g + 1) * 128, go:go + rstep * WO * T
                            ],
                            in_=otile[:, rs * WO * T:(rs + rstep) * WO * T],
                        )
                    del ktiles[blk]
    nc.compile()
    return nc


def _get_nc():
    if "nc" not in _CACHE:
        _CACHE["nc"] = _build_nc()
    return _CACHE["nc"]


def _make_in_maps(key_map, query_map):
    kflat = np.ascontiguousarray(key_map.reshape(NPLANES, H, W))
    qflat = np.ascontiguousarray(query_map.reshape(NPLANES, H, W))
    in_maps = []
    for i in range(NCORES):
        r0 = ROWS * i
        kshard = np.zeros((NPLANES, KR, W), np.float32)
        nrows = min(KR, H - r0)
        kshard[:, :nrows] = kflat[:, r0:r0 + nrows]
        qshard = np.zeros((NPLANES, ROWS, W), np.float32)
        qrows = min(ROWS, H - (r0 + 1))
        qshard[:, :qrows] = qflat[:, r0 + 1:r0 + 1 + qrows]
        in_maps.append({
            "key": kshard.reshape(NPLANES, KR * W),
            "query": qshard.reshape(NPLANES, ROWS * W),
        })
    return in_maps


def run_spmd(key_map, query_map, trace=False, **kwargs):
    from concourse.bass_utils import run_bass_kernel_spmd

    nc = _get_nc()
    in_maps = _make_in_maps(key_map, query_map)
    res = run_bass_kernel_spmd(
        nc, in_maps, core_ids=list(range(NCORES)), trace=trace, **kwargs
    )
    outs = [res.results[i]["out"].reshape(NPLANES, ROWS, WO, K, K)
            for i in range(NCORES)]
    full = np.concatenate(outs, axis=1)[:, :HO]
    return full.reshape(B, C, HO * WO, K, K), res


def kernel(key_map, query_map, k, stride):
    assert int(k) == K and int(stride) == 1
    key_map = np.asarray(key_map, dtype=np.float32)
    query_map = np.asarray(query_map, dtype=np.float32)
    out, _ = run_spmd(key_map, query_map, trace=False)
    return out
